# revision 1
# baseline (speedup 1.0000x reference)
"""Trainium2 Bass kernel for nn_Network_21998822490747 (embedding -> tiny LSTM -> vocab projection).

Strategy (8 NeuronCores, full inputs in / full output out):
  * Time-shard the T=4096 sequence: core c owns rows [c*512, (c+1)*512).
  * The LSTM recurrence is contractive (forget gate sigma(|x|<~1) <= 0.7), so each
    core runs S=32 parallel "streams" (time-chunks of L=16 steps) that each start
    W=48 steps early from zero state; after the warmup the state matches the exact
    scan to below fp32 noise (validated: max |h| err ~5e-8 for W>=24; W=32 used).
    Streams are vectorized along the SBUF free dimension, so one scan step is 7
    engine instructions covering all 32 streams.
  * All gate activations use a single tanh per step:
      sigmoid(x) = 0.5*(tanh(x/2)+1), handled with a per-partition scale vector
      and (t+1)-style fused scalar_tensor_tensor ops; state is kept as
      C=2c, h2=2h, with the 0.5 factors folded into w_hh and W_out host-side.
  * The memory-bound phase (this problem's target regime) is the [512,10+1] @
    [11, 50257] logits matmul per core: float32r matmuls (full PE rate) into
    PSUM, drained to SBUF alternating DVE/ACT, DMA'd to HBM at ~360GB/s/core.
  * The embedding gather runs on-device via one indirect DMA (2048 rows/core,
    incl. warmup rows) from the full table in device DRAM; an appended row V
    (least-squares solution of w_ih @ v = -(b_ih+b_hh)) makes out-of-range
    warmup steps exact no-ops so stream 0 starts from the true zero state.
"""

import os
import sys
import time

for _p in ("/opt/trn_rl_repo", "/root/.axon_site/_ro/trn_rl_repo"):
    if os.path.isdir(_p) and _p not in sys.path:
        sys.path.insert(0, _p)

import numpy as np

import concourse.bass as bass
import concourse.bacc as bacc
import concourse.mybir as mybir
import concourse.tile as tile
from concourse.bass import ts
from concourse.masks import make_identity

# Problem shapes
T, V, E, H, O = 4096, 128000, 256, 10, 50257
NCORES = 8
ROWS = T // NCORES        # 512 output rows per core

# Scan decomposition
S = 64                    # parallel streams per core
L = ROWS // S             # 16 real steps per stream
W = 16                    # warmup steps per stream
STEPS = L + W             # 64
NR = S * STEPS            # 2048 gathered rows per core
CB = NR // 128            # 16 gather column-blocks

# Logits tiling
OPAD = 51200              # O padded to 2 halves x 50 x 512
NQ = 2                    # wout partition groups (PE operand base must be 0/32/64)
QD = OPAD // NQ           # 25600
OC = QD // 512            # 50 moving chunks per half
STG = 5120                # staging tile columns per DMA batch
CPS = STG // 512          # psum chunks per staging tile

f32 = mybir.dt.float32
f32r = mybir.dt.float32r
i32 = mybir.dt.int32
AF = mybir.ActivationFunctionType
AL = mybir.AluOpType

GATE_PERM = np.r_[0:10, 10:20, 30:40, 20:30]   # [i, f, o, g] row order


def _tile_kernel(tc, nc, emb, idx, wihT, whhT, b40, wout, out, embg=None,
                 variant="gather16"):
    with (
        tc.tile_pool(name="const", bufs=1) as cpool,
        tc.tile_pool(name="work", bufs=1) as wpool,
    ):
        wih_sb = cpool.tile([128, 80], f32, tag="wih")
        whh_sb = cpool.tile([H, 40], f32, tag="whh")
        b40_sb = cpool.tile([40, 1], f32, tag="b40")
        ident = cpool.tile([128, 128], f32, tag="ident")
        wout_sb = cpool.tile([128, QD], f32r, tag="wout")

        nc.sync.dma_start(wih_sb[:, 0:40], wihT[0:128, :])
        nc.sync.dma_start(wih_sb[:, 40:80], wihT[128:256, :])
        nc.sync.dma_start(whh_sb[:], whhT[:])
        nc.sync.dma_start(b40_sb[:], b40[:])
        nc.sync.dma_start(wout_sb[:], wout[:])
        make_identity(nc, ident[:])

        # ---- gather + transpose + xg (gather tiles freed before logits)
        # xg32: streams-on-partitions layout, step t at cols [t*40, (t+1)*40)
        xg32 = wpool.tile([S, STEPS * 40], f32, tag="xg32")
        with tc.tile_pool(name="gath", bufs=1) as gpool:
            # gather 2048 embedding rows (incl. warmup rows)
            emb_raw = gpool.tile([128, CB * E], f32, tag="raw")
            if variant == "hostgather":
                nc.sync.dma_start(emb_raw[:], embg[:])
            elif variant == "gather1":
                idx_sb = cpool.tile([128, CB], i32, tag="idx")
                nc.sync.dma_start(idx_sb[:], idx[:])
                nc.gpsimd.indirect_dma_start(
                    out=emb_raw[:].rearrange("p (c e) -> p c e", e=E),
                    out_offset=None,
                    in_=emb[:, :],
                    in_offset=bass.IndirectOffsetOnAxis(ap=idx_sb[:, :], axis=0),
                )
            else:  # gather16: one [128,1]-offset indirect DMA per column block
                idx_sb = cpool.tile([128, CB], i32, tag="idx")
                nc.sync.dma_start(idx_sb[:], idx[:])
                for c in range(CB):
                    nc.gpsimd.indirect_dma_start(
                        out=emb_raw[:, c * E:(c + 1) * E],
                        out_offset=None,
                        in_=emb[:, :],
                        in_offset=bass.IndirectOffsetOnAxis(
                            ap=idx_sb[:, c:c + 1], axis=0),
                    )

            # transpose to emb^T layout [E, NR] (two 128-row halves)
            embT0 = gpool.tile([128, NR], f32, tag="embT0")
            embT1 = gpool.tile([128, NR], f32, tag="embT1")
            embTv = [embT0, embT1]
            with tc.tile_pool(name="pst", bufs=4, space="PSUM") as pst:
                for c in range(CB):
                    for e2 in range(2):
                        ps = pst.tile([128, 128], f32, tag="tp")
                        base = c * E + e2 * 128
                        nc.tensor.transpose(ps[:], emb_raw[:, base:base + 128],
                                            ident[:])
                        dst = embTv[e2][:].rearrange("q (p c) -> q p c", c=CB)[:, :, c]
                        nc.vector.tensor_copy(dst, ps[:])

            # xg40 = emb @ w_ih^T + bias (pre-scaled on host) -> [40, NR]
            xg40 = gpool.tile([40, NR], f32, tag="xg40")
            with tc.tile_pool(name="psx", bufs=2, space="PSUM") as psx:
                for n in range(NR // 512):
                    ps = psx.tile([40, 512], f32, tag="xg")
                    nc.tensor.matmul(ps[:], lhsT=wih_sb[:, 0:40],
                                     rhs=embT0[:, ts(n, 512)],
                                     start=True, stop=False)
                    nc.tensor.matmul(ps[:], lhsT=wih_sb[:, 40:80],
                                     rhs=embT1[:, ts(n, 512)],
                                     start=False, stop=True)
                    nc.scalar.activation(xg40[:, ts(n, 512)], ps[:], AF.Identity,
                                         bias=b40_sb[:, 0:1], scale=1.0)

            # transpose xg40 step-blocks [40, 32] -> xg32 blocks [32, 40]
            with tc.tile_pool(name="psx2", bufs=4, space="PSUM") as psx2:
                for t in range(STEPS):
                    ps = psx2.tile([S, 40], f32, tag="xt")
                    nc.tensor.transpose(ps[:], xg40[:, ts(t, S)], ident[0:40, 0:40])
                    nc.vector.tensor_copy(xg32[:, ts(t, 40)], ps[:])

        # ---- vectorized scan: 64 steps x 32 streams (streams on partitions)
        # th free-col layout: 0:40 tanh(gates i,f,o,g) | 40:50 C(=2c) | 50:60 tanh(c)
        hs = wpool.tile([11, (STEPS + 1) * S], f32, tag="hs")   # hT history + ones row
        th = wpool.tile([S, 60], f32, tag="th")
        gt = wpool.tile([S, 40], f32, tag="gt")
        uv = wpool.tile([S, 20], f32, tag="uv")
        h2 = wpool.tile([S, H], f32, tag="h2")
        nc.vector.memset(hs[:, :], 1.0)          # row 10 stays 1.0 (bias row)
        nc.vector.memset(th[:, 40:50], 0.0)      # C = 2c state
        nc.vector.memset(h2[:, :], 0.0)
        with (
            tc.tile_pool(name="psm", bufs=2, space="PSUM") as psm,
            tc.tile_pool(name="pst2", bufs=2, space="PSUM") as pst2,
        ):
            for t in range(STEPS + 1):
                # hT(t) = h2(t-1)^T  -> also the hs history used by logits
                pst_ = pst2.tile([H, S], f32, tag="ht")
                nc.tensor.transpose(pst_[:], h2[:, :], ident[0:S, 0:S])
                nc.vector.tensor_copy(hs[0:10, ts(t, S)], pst_[:])
                if t == STEPS:
                    break
                ps = psm.tile([S, 40], f32, tag="mv")
                nc.tensor.matmul(ps[:], lhsT=hs[0:10, ts(t, S)], rhs=whh_sb[:],
                                 start=True, stop=True)
                nc.vector.scalar_tensor_tensor(gt[:, :], ps[:], 1.0,
                                               xg32[:, ts(t, 40)], AL.mult, AL.add)
                nc.scalar.activation(th[:, 0:40], gt[:, :], AF.Tanh)
                # u = (th_i+1)*th_g ; v = (th_f+1)*C   (one fused op)
                nc.vector.scalar_tensor_tensor(uv[:, :], th[:, 0:20], 1.0,
                                               th[:, 30:50], AL.add, AL.mult)
                nc.vector.scalar_tensor_tensor(th[:, 40:50], uv[:, 10:20], 0.5,
                                               uv[:, 0:10], AL.mult, AL.add)
                nc.scalar.activation(th[:, 50:60], th[:, 40:50], AF.Tanh, scale=0.5)
                nc.vector.scalar_tensor_tensor(h2[:, :], th[:, 20:30], 1.0,
                                               th[:, 50:60], AL.add, AL.mult)

        # ---- logits: [11, 128]^T @ [11, 512] f32r matmuls, drain, DMA out
        hs_r = hs[:].rearrange("p (t s) -> p s t", s=S)    # [11, S, STEPS+1]
        with (
            tc.tile_pool(name="psl", bufs=8, space="PSUM") as psl,
            tc.tile_pool(name="stage", bufs=3) as stpool,
            tc.tile_pool(name="statp", bufs=2) as statpool,
        ):
            SPB = 128 // L           # streams per 128-row block
            for blk in range(ROWS // 128):
                s0 = blk * SPB
                # PE needs stationary+moving at the same base partition; wout
                # lives in NQ partition groups, so replicate the tiny hs block
                # into each group of statq.
                statq = statpool.tile([128, 128], f32r, tag="statq")
                for q in range(NQ):
                    nc.vector.tensor_copy(
                        statq[64 * q:64 * q + 11, :].rearrange(
                            "p (a b) -> p a b", b=L),
                        hs_r[0:11, s0:s0 + SPB, W + 1:W + 1 + L])
                for q in range(NQ):
                    stat = statq[64 * q:64 * q + 11, :]
                    stage = None
                    for oc in range(OC):
                        ps = psl.tile([128, 512], f32, tag="lg")
                        nc.tensor.matmul(
                            ps[:], lhsT=stat,
                            rhs=wout_sb[64 * q:64 * q + 11, ts(oc, 512)],
                            start=True, stop=True)
                        if oc % CPS == 0:
                            stage = stpool.tile([128, STG], f32, tag="stg")
                        if oc & 1:
                            nc.scalar.copy(stage[:, ts(oc % CPS, 512)], ps[:])
                        else:
                            nc.vector.tensor_copy(stage[:, ts(oc % CPS, 512)], ps[:])
                        if oc % CPS == CPS - 1:
                            col = q * QD + (oc // CPS) * STG
                            nc.sync.dma_start(
                                out[ts(blk, 128), col:col + STG], stage[:])


def build_program_real(variant="gather16"):
    nc = bacc.Bacc("TRN2", target_bir_lowering=False, debug=False,
                   enable_asserts=False)
    emb_ap = idx_ap = embg_ap = None
    if variant == "hostgather":
        embg_ap = nc.dram_tensor("embg", [128, CB * E], f32,
                                 kind="ExternalInput").ap()
    else:
        emb_ap = nc.dram_tensor("emb", [V + 1, E], f32, kind="ExternalInput").ap()
        idx_ap = nc.dram_tensor("idx", [128, CB], i32, kind="ExternalInput").ap()
    wih_d = nc.dram_tensor("wihT", [E, 40], f32, kind="ExternalInput")
    whh_d = nc.dram_tensor("whhT05", [H, 40], f32, kind="ExternalInput")
    b40_d = nc.dram_tensor("bias40", [40, 1], f32, kind="ExternalInput")
    wout_d = nc.dram_tensor("wout", [128, QD], f32r, kind="ExternalInput")
    out_d = nc.dram_tensor("out", [ROWS, OPAD], f32, kind="ExternalOutput")

    with tile.TileContext(nc) as tc:
        _tile_kernel(tc, nc, emb_ap, idx_ap, wih_d.ap(), whh_d.ap(),
                     b40_d.ap(), wout_d.ap(), out_d.ap(), embg=embg_ap,
                     variant=variant)
    nc.compile()
    return nc


def prep_host(inputs):
    """Shared (core-independent) prepped arrays + per-core index tables."""
    x = np.asarray(inputs["x"]).astype(np.int64)
    embedding = np.asarray(inputs["embedding"], dtype=np.float32)
    w_ih = np.asarray(inputs["w_ih"], dtype=np.float32)
    w_hh = np.asarray(inputs["w_hh"], dtype=np.float32)
    b_ih = np.asarray(inputs["b_ih"], dtype=np.float32)
    b_hh = np.asarray(inputs["b_hh"], dtype=np.float32)
    W_out = np.asarray(inputs["W_out"], dtype=np.float32)
    b_out = np.asarray(inputs["b_out"], dtype=np.float32)

    p = GATE_PERM
    # gate scale: sigmoid(x) = 0.5*(tanh(x/2)+1) -> scale i,f,o preacts by 0.5,
    # folded into w_ih / bias; w_hh additionally gets the h2=2h factor (x0.5).
    gsc = np.concatenate([np.full(30, 0.5), np.ones(10)]).astype(np.float32)
    w_ih_p = w_ih[p] * gsc[:, None]
    bias40 = ((b_ih + b_hh)[p] * gsc).astype(np.float32)
    whh05 = (w_hh[p].T * (0.5 * gsc)[None, :]).astype(np.float32)   # [10, 40]
    wihT = np.ascontiguousarray(w_ih_p.T).astype(np.float32)        # [256, 40]

    # Padding row V: w_ih @ v = -(b_ih + b_hh)  => xg row == 0 for padded steps
    v, *_ = np.linalg.lstsq(w_ih.astype(np.float64), -(b_ih + b_hh).astype(np.float64),
                            rcond=None)
    emb_aug = np.concatenate([embedding, v[None, :].astype(np.float32)], axis=0)

    woutp = np.zeros((128, QD), np.float32)
    Wt = np.zeros((OPAD, H), np.float32)
    Wt[:O] = 0.5 * W_out
    bo = np.zeros(OPAD, np.float32)
    bo[:O] = b_out
    for q in range(NQ):
        woutp[64 * q:64 * q + 10, :] = Wt[q * QD:(q + 1) * QD].T
        woutp[64 * q + 10, :] = bo[q * QD:(q + 1) * QD]

    idx_cores = []
    embg_cores = []
    for c in range(NCORES):
        j = np.arange(NR)
        t = j // S
        s = j % S
        g_r = c * ROWS + s * L - W + t
        val = np.where(g_r < 0, V, x[np.clip(g_r, 0, T - 1)])
        # tile position (p, cb) holds gather row j = p*CB + cb
        idx_cores.append(val.reshape(128, CB).astype(np.int32))
        embg_cores.append(emb_aug[val].reshape(128, CB * E).astype(np.float32))

    shared = {
        "emb": emb_aug,
        "wihT": wihT,
        "whhT05": whh05,
        "bias40": bias40.reshape(40, 1),
        "wout": woutp,
    }
    return shared, idx_cores, embg_cores


def in_maps_for(inputs):
    shared, idx_cores, embg_cores = prep_host(inputs)
    return [{**shared, "idx": idx_cores[c], "embg": embg_cores[c]}
            for c in range(NCORES)]


_EXEC_CACHE = {}


def _get_exec(variant="gather16"):
    """Build (once) the compiled 8-core PJRT executable and metadata."""
    if variant in _EXEC_CACHE:
        return _EXEC_CACHE[variant]

    import jax
    from jax.sharding import Mesh, PartitionSpec, NamedSharding
    try:
        from jax.experimental.shard_map import shard_map
    except ImportError:
        from jax import shard_map
    from concourse import bass2jax

    bass2jax.install_neuronx_cc_hook()
    nc = build_program_real(variant)

    pname = nc.partition_id_tensor.name if nc.partition_id_tensor else None
    in_names, out_names, out_avals = [], [], []
    for alloc in nc.m.functions[0].allocations:
        if not isinstance(alloc, mybir.MemoryLocationSet):
            continue
        name = alloc.memorylocations[0].name
        if alloc.kind == "ExternalInput":
            if name != pname:
                in_names.append(name)
        elif alloc.kind == "ExternalOutput":
            out_names.append(name)
            out_avals.append(jax.core.ShapedArray(
                tuple(alloc.tensor_shape), mybir.dt.np(alloc.dtype)))
    n_params = len(in_names)
    all_names = in_names + out_names + ([pname] if pname else [])

    def _body(*args):
        operands = list(args)
        if pname is not None:
            operands.append(bass2jax.partition_id_tensor())
        outs = bass2jax._bass_exec_p.bind(
            *operands,
            out_avals=tuple(out_avals),
            in_names=tuple(all_names),
            out_names=tuple(out_names),
            lowering_input_output_aliases=(),
            sim_require_finite=False,
            sim_require_nnan=False,
            nc=nc,
        )
        return tuple(outs)

    devices = jax.devices()[:NCORES]
    mesh = Mesh(np.asarray(devices), ("core",))
    spec_in = (PartitionSpec("core"),) * (n_params + len(out_names))
    spec_out = (PartitionSpec("core"),) * len(out_names)
    donate = tuple(range(n_params, n_params + len(out_names)))
    fn = jax.jit(
        shard_map(_body, mesh=mesh, in_specs=spec_in, out_specs=spec_out,
                  check_rep=False),
        donate_argnums=donate, keep_unused=True)

    res = {
        "jax": jax, "mesh": mesh, "NamedSharding": NamedSharding,
        "PartitionSpec": PartitionSpec, "fn": fn, "nc": nc,
        "in_names": in_names, "out_names": out_names, "out_avals": out_avals,
        "devices": devices,
    }
    _EXEC_CACHE[variant] = res
    return res


def _place_inputs(ex, in_maps):
    """Transfer per-core input shards to the 8 devices, return global arrays."""
    jax = ex["jax"]
    NamedSharding, PartitionSpec = ex["NamedSharding"], ex["PartitionSpec"]
    sharding = NamedSharding(ex["mesh"], PartitionSpec("core"))
    placed = []
    for name in ex["in_names"]:
        shards = [np.asarray(in_maps[c][name]) for c in range(NCORES)]
        per_dev = [jax.device_put(s, d) for s, d in zip(shards, ex["devices"])]
        gshape = (NCORES * shards[0].shape[0],) + shards[0].shape[1:]
        placed.append(jax.make_array_from_single_device_arrays(
            gshape, sharding, per_dev))
    jax.block_until_ready(placed)
    return placed, sharding


def _zero_outs(ex, sharding):
    import jax.numpy as jnp
    outs = []
    for av in ex["out_avals"]:
        gshape = (NCORES * av.shape[0],) + av.shape[1:]
        outs.append(jnp.zeros(gshape, av.dtype, device=sharding))
    ex["jax"].block_until_ready(outs)
    return outs


def run_hw(inputs, time_iters=0, variant=None):
    """Run on the 8 NeuronCores. Returns (full_output, wall_times_s)."""
    if variant is None:
        variant = os.environ.get("KERNEL_VARIANT", "gather16")
    ex = _get_exec(variant)
    jax = ex["jax"]
    in_maps = in_maps_for(inputs)
    placed, sharding = _place_inputs(ex, in_maps)

    zouts = _zero_outs(ex, sharding)
    res = ex["fn"](*placed, *zouts)
    jax.block_until_ready(res)
    out_global = np.asarray(res[0])          # [8*512, OPAD]

    times = []
    for _ in range(time_iters):
        zouts = _zero_outs(ex, sharding)
        t0 = time.perf_counter()
        r = ex["fn"](*placed, *zouts)
        jax.block_until_ready(r)
        times.append(time.perf_counter() - t0)

    full = out_global[:, :O].reshape(T, 1, O).astype(np.float32)
    return full, times


def kernel(**inputs):
    out, _ = run_hw(inputs, time_iters=0)
    return out


# ---------------------------------------------------------------- dev helpers

def sim_check(inputs, core=0, variant="gather16"):
    """Run core `core`'s program in CoreSim, return its [512, OPAD] output."""
    from concourse.bass_interp import CoreSim
    nc = build_program_real(variant)
    sim = CoreSim(nc, trace=False, require_finite=False, require_nnan=False)
    in_maps = in_maps_for(inputs)
    for name, arr in in_maps[core].items():
        try:
            sim.tensor(name)[:] = arr
        except KeyError:
            pass
    sim.simulate(check_with_hw=False)
    return np.array(sim.tensor("out"))


def timeline(variant="gather16"):
    from concourse.timeline_sim import TimelineSim
    nc = build_program_real(variant)
    tl = TimelineSim(nc, trace=False)
    tl.simulate()
    return tl


def probe_floor(iters=5):
    """Wall-time floor of the 8-core dispatch path using a trivial NEFF."""
    import jax
    from jax.sharding import Mesh, PartitionSpec, NamedSharding
    try:
        from jax.experimental.shard_map import shard_map
    except ImportError:
        from jax import shard_map
    from concourse import bass2jax
    bass2jax.install_neuronx_cc_hook()

    nc = bacc.Bacc("TRN2", target_bir_lowering=False, debug=False,
                   enable_asserts=False)
    pin = nc.dram_tensor("pin", [128, 128], f32, kind="ExternalInput")
    pout = nc.dram_tensor("pout", [128, 128], f32, kind="ExternalOutput")
    with tile.TileContext(nc) as tc:
        with tc.tile_pool(name="p", bufs=1) as pool:
            t = pool.tile([128, 128], f32, tag="t")
            nc.sync.dma_start(t[:], pin.ap()[:])
            nc.sync.dma_start(pout.ap()[:], t[:])
    nc.compile()

    pname = nc.partition_id_tensor.name if nc.partition_id_tensor else None
    all_names = ["pin", "pout"] + ([pname] if pname else [])

    def _body(a, z):
        ops = [a, z]
        if pname is not None:
            ops.append(bass2jax.partition_id_tensor())
        return tuple(bass2jax._bass_exec_p.bind(
            *ops, out_avals=(jax.core.ShapedArray((128, 128), np.float32),),
            in_names=tuple(all_names), out_names=("pout",),
            lowering_input_output_aliases=(),
            sim_require_finite=False, sim_require_nnan=False, nc=nc))

    devices = jax.devices()[:NCORES]
    mesh = Mesh(np.asarray(devices), ("core",))
    sharding = NamedSharding(mesh, PartitionSpec("core"))
    fn = jax.jit(shard_map(_body, mesh=mesh,
                           in_specs=(PartitionSpec("core"),) * 2,
                           out_specs=(PartitionSpec("core"),),
                           check_rep=False), keep_unused=True)
    import jax.numpy as jnp
    a = jax.device_put(np.zeros((NCORES * 128, 128), np.float32), sharding)
    z = jnp.zeros((NCORES * 128, 128), np.float32, device=sharding)
    jax.block_until_ready([a, z])
    r = fn(a, z); jax.block_until_ready(r)   # warm

    def timed(reps):
        best = float("inf")
        for _ in range(iters):
            t0 = time.perf_counter()
            r = None
            for _ in range(reps):
                r = fn(a, z)
            jax.block_until_ready(r)
            best = min(best, time.perf_counter() - t0)
        return best

    w1 = timed(1)
    wk = timed(50)
    return (wk - w1) / 49.0, wk, w1

def run_hw_async(inputs, k=50, iters=3, variant="gather16"):
    """Per-exec time via async pipelining: submit k executions without
    intermediate blocking; marginal cost per call ~= device exec time if the
    runtime queues them. Returns (per_exec_s, wall_k, wall_1)."""
    import jax
    from jax.sharding import PartitionSpec
    try:
        from jax.experimental.shard_map import shard_map
    except ImportError:
        from jax import shard_map
    from concourse import bass2jax
    ex = _get_exec(variant)
    nc = ex["nc"]
    pname = nc.partition_id_tensor.name if nc.partition_id_tensor else None
    in_names, out_names, out_avals = ex["in_names"], ex["out_names"], ex["out_avals"]
    all_names = in_names + out_names + ([pname] if pname else [])

    def _body(*args):
        ops = list(args)
        if pname is not None:
            ops.append(bass2jax.partition_id_tensor())
        return tuple(bass2jax._bass_exec_p.bind(
            *ops, out_avals=tuple(out_avals), in_names=tuple(all_names),
            out_names=tuple(out_names), lowering_input_output_aliases=(),
            sim_require_finite=False, sim_require_nnan=False, nc=nc))

    nin = len(in_names) + len(out_names)
    fn = jax.jit(shard_map(_body, mesh=ex["mesh"],
                           in_specs=(PartitionSpec("core"),) * nin,
                           out_specs=(PartitionSpec("core"),) * len(out_names),
                           check_rep=False), keep_unused=True)  # no donation

    in_maps = in_maps_for(inputs)
    placed, sharding = _place_inputs(ex, in_maps)
    zouts = _zero_outs(ex, sharding)
    r = fn(*placed, *zouts); jax.block_until_ready(r)   # warm

    def timed(reps):
        best = float("inf")
        for _ in range(iters):
            t0 = time.perf_counter()
            r = None
            for _ in range(reps):
                r = fn(*placed, *zouts)
            jax.block_until_ready(r)
            best = min(best, time.perf_counter() - t0)
        return best

    w1 = timed(1)
    wk = timed(k)
    return (wk - w1) / (k - 1), wk, w1



# revision 26
# speedup vs baseline: 3.2927x; 3.2927x over previous
"""Trainium2 Bass kernel for nn_Network_21998822490747 (embedding -> tiny LSTM -> vocab projection).

Strategy (8 NeuronCores, full inputs in / full output out):
  * Time-shard the T=4096 sequence: core c owns rows [c*512, (c+1)*512).
  * Each core gathers its 512 tokens plus W=8 warmup tokens (the recurrence is
    contractive, forget gate ~0.5, so a zero initial state W steps early
    matches the exact scan far below the accuracy target).
  * The LSTM itself runs as K=3 fixed-point sweeps instead of a per-step scan:
    gate pre-activations for ALL rows come from one pair of full-rate f32r
    matmuls (plus the tiny w_hh @ h term re-computed per sweep from the
    previous sweep's h), the cell recurrence c = f*c + i*g is ONE
    tensor_tensor_scan instruction along the free axis, and h = o * tanh(c).
    w_hh is tiny so the h-feedback converges at ~0.2x error per sweep
    (validated end-to-end: rel err 2.7e-3 vs 2e-2 budget).
  * Gates live at partition bases 0/32/64/96 (i/f/o/g) so every engine slice
    starts on a legal SBUF quadrant.
  * The memory-bound phase is the [512,11] @ [11, 50257] logits matmul per
    core: f32r matmuls (full PE rate) into 2-bank PSUM tiles, drained to fp16
    staging tiles by a DVE/ACT/GpSimd rotation, and DMA'd to HBM at half the
    bytes of an f32 store. fp16 logits keep rel err ~2e-4 << 2e-2.
  * The embedding gather runs on-device via indirect DMA from the full table
    in device DRAM; an appended row V (least-squares solution of
    w_ih @ v = -(b_ih+b_hh)) makes out-of-range warmup rows exact no-ops.
"""

import os
import sys
import time

for _p in ("/opt/trn_rl_repo", "/root/.axon_site/_ro/trn_rl_repo"):
    if os.path.isdir(_p) and _p not in sys.path:
        sys.path.insert(0, _p)

import numpy as np

import concourse.bass as bass
import concourse.bacc as bacc
import concourse.mybir as mybir
import concourse.tile as tile
from concourse.bass import ts
from concourse.masks import make_identity

# Problem shapes
T, V, E, H, O = 4096, 128000, 256, 10, 50257
NCORES = 8
ROWS = T // NCORES        # 512 output rows per core

# Scan decomposition
W = 8                     # warmup rows (zero-state start, contractive decay)
NSWEEP = 3                # fixed-point sweeps for the w_hh @ h feedback
NUSE = W + ROWS           # 520 live rows
CB = 5                    # gather column-blocks of 128 rows
NR = CB * 128             # 640 gathered rows per core (tail padded)

# Gate partition layout: i at 0:10, f at 32:42, o at 64:74, g at 96:106
GP = 106
GOFF = (0, 32, 64, 96)    # i, f, o, g base partitions

# Logits tiling
QD = 25600                # columns per wout partition group
NQ = 2                    # partition groups (stationary base must be 0/64)
CW = 512                  # psum chunk width (one bank)
STG = 12288               # staging tile columns per DMA batch (12 psum pairs)

f32 = mybir.dt.float32
f32r = mybir.dt.float32r
fp16 = mybir.dt.float16
i32 = mybir.dt.int32
AF = mybir.ActivationFunctionType
AL = mybir.AluOpType

# PyTorch gate order (i,f,g,o) -> ours (i,f,o,g)
GATE_PERM = np.r_[0:10, 10:20, 30:40, 20:30]

# per-q chunk schedule: (col0, width) within the q's QD-wide region
def _chunks_for_q(q):
    width = QD if q == 0 else O - QD          # 25600 / 24657
    out = []
    c0 = 0
    while c0 < width:
        w = min(CW, width - c0)
        w += w % 2          # f32r matmul needs an even moving size; region
        out.append((c0, w))  # is zero-padded past O so +1 col is harmless
        c0 += w
    return out


def _tile_kernel(tc, nc, emb, idx, wih, whh, b106, wout, out,
                 drain_cycle=("v", "a", "v", "a", "p"), dbg=None,
                 phases=("front", "sweep", "logits")):
    with tile.ExitStack() as stack:
        cpool = stack.enter_context(tc.tile_pool(name="const", bufs=1))
        wpool = stack.enter_context(tc.tile_pool(name="work", bufs=1))

        wih_sb = cpool.tile([128, 2 * GP], f32, tag="wih")
        whh_sb = cpool.tile([H, GP], f32r, tag="whh")
        b106_sb = cpool.tile([GP, 1], f32, tag="b106")
        ident = cpool.tile([128, 128], f32, tag="ident")
        wout_sb = cpool.tile([128, QD], f32r, tag="wout")
        idx_sb = cpool.tile([128, CB], i32, tag="idx")

        nc.sync.dma_start(wih_sb[:, 0:GP], wih[0:128, :])
        nc.sync.dma_start(wih_sb[:, GP:2 * GP], wih[128:256, :])
        nc.sync.dma_start(whh_sb[:], whh[:])
        nc.sync.dma_start(b106_sb[:], b106[:])
        nc.sync.dma_start(wout_sb[0:11, :], wout[0:11, :])
        nc.sync.dma_start(wout_sb[64:75, :], wout[11:22, :])
        nc.sync.dma_start(idx_sb[:], idx[:])
        make_identity(nc, ident[:])

        xg = wpool.tile([GP, NR], f32, tag="xg")
        hsb = wpool.tile([11, NR], f32, tag="hsb")
        zcol = wpool.tile([H, 1], f32, tag="zcol")
        nc.vector.memset(hsb[:], 1.0)      # row 10 stays 1.0 (bias row)
        nc.vector.memset(zcol[:], 0.0)

        # ---- gather + transpose + xg (gate pre-acts from the embedding)
        if "front" not in phases:
            nc.vector.memset(xg[:], 0.01)
        else:
            with (
                tc.tile_pool(name="gath", bufs=1) as gpool,
                tc.tile_pool(name="pst", bufs=4, space="PSUM") as pst,
                tc.tile_pool(name="psx", bufs=1, space="PSUM") as psx,
            ):
                emb_raw = gpool.tile([128, CB * E], f32, tag="raw")
                embT0 = gpool.tile([128, NR], f32, tag="embT0")
                embT1 = gpool.tile([128, NR], f32, tag="embT1")
                for c in range(CB):
                    nc.gpsimd.indirect_dma_start(
                        out=emb_raw[:, c * E:(c + 1) * E],
                        out_offset=None,
                        in_=emb[:, :],
                        in_offset=bass.IndirectOffsetOnAxis(
                            ap=idx_sb[:, c:c + 1], axis=0),
                    )
                    for e2, dst in ((0, embT0), (1, embT1)):
                        ps = pst.tile([128, 128], f32, tag="tp")
                        nc.tensor.transpose(
                            ps[:], emb_raw[:, c * E + e2 * 128:
                                           c * E + e2 * 128 + 128],
                            ident[:])
                        nc.vector.tensor_copy(dst[:, ts(c, 128)], ps[:])

                # xg = emb @ w_ih^T (+bias on drain): [106, 640] 2-bank psum
                psxg = psx.tile([GP, NR], f32, tag="xgp")
                for lo, hi in ((0, 512), (512, NR)):
                    nc.tensor.matmul(psxg[:, lo:hi], lhsT=wih_sb[:, 0:GP],
                                     rhs=embT0[:, lo:hi],
                                     start=True, stop=False)
                    nc.tensor.matmul(psxg[:, lo:hi], lhsT=wih_sb[:, GP:2 * GP],
                                     rhs=embT1[:, lo:hi],
                                     start=False, stop=True)
                nc.scalar.activation(xg[:], psxg[:], AF.Identity,
                                     bias=b106_sb[:, 0:1], scale=1.0)

        # ---- fixed-point sweeps (all ops on the NUSE live columns)
        NU = NUSE
        with (
            tc.tile_pool(name="swp", bufs=2) as spool,
            tc.tile_pool(name="psg", bufs=2, space="PSUM") as psg,
        ):
            h_prev = None
            nsw = NSWEEP if "sweep" in phases else 1
            for k in range(nsw):
                if k == 0:
                    gsrc = xg            # h == 0: gates come straight from xg
                else:
                    # PE accumulation ignores DVE-prefilled PSUM on real HW,
                    # so matmul normally and add xg with one DVE op instead.
                    ps = psg.tile([GP, NU], f32, tag="g")
                    nc.tensor.matmul(ps[:, 0:512], lhsT=whh_sb[:],
                                     rhs=h_prev[:, 0:512],
                                     start=True, stop=True)
                    nc.tensor.matmul(ps[:, 512:NU], lhsT=whh_sb[:],
                                     rhs=h_prev[:, 512:NU],
                                     start=True, stop=True)
                    pre = spool.tile([GP, NU], f32, tag=f"pre{k}")
                    nc.vector.tensor_tensor(pre[:], ps[:], xg[:, 0:NU],
                                            AL.add)
                    gsrc = pre
                # separate base-0 tiles per gate: the compiler requires both
                # SBUF inputs of a tensor-tensor op to share a base partition
                Si = spool.tile([H, NU], f32, tag="Si")
                Sf = spool.tile([H, NU], f32, tag="Sf")
                So = spool.tile([H, NU], f32, tag="So")
                G = spool.tile([H, NU], f32, tag="G")
                u = spool.tile([H, NU], f32, tag="u")
                C = spool.tile([H, NU], f32, tag="C")
                TC = spool.tile([H, NU], f32, tag="TC")
                nc.scalar.activation(G[:], gsrc[96:106, 0:NU], AF.Tanh)
                nc.scalar.activation(Si[:], gsrc[0:10, 0:NU], AF.Sigmoid)
                nc.scalar.activation(Sf[:], gsrc[32:42, 0:NU], AF.Sigmoid)
                nc.scalar.activation(So[:], gsrc[64:74, 0:NU], AF.Sigmoid)
                nc.vector.tensor_tensor(u[:], Si[:], G[:], AL.mult)
                nc.vector.tensor_tensor_scan(C[:], Sf[:], u[:], 0.0,
                                             AL.mult, AL.add)
                nc.scalar.activation(TC[:], C[:], AF.Tanh)
                if k < nsw - 1:
                    h_new = spool.tile([H, NU], f32r, tag=f"h{k}")
                    nc.vector.tensor_copy(h_new[:, 0:1], zcol[:])
                    nc.vector.tensor_tensor(h_new[:, 1:NU], So[:, 0:NU - 1],
                                            TC[:, 0:NU - 1], AL.mult)
                    h_prev = h_new
                else:
                    nc.vector.tensor_tensor(hsb[0:10, 0:NU], So[:], TC[:],
                                            AL.mult)

        if dbg is not None:
            nc.sync.dma_start(dbg["xg"][:], xg[:])
            nc.sync.dma_start(dbg["hsb"][:], hsb[:])

        if "logits" not in phases:
            return
        # ---- logits: [11, 128]^T @ [11, 512] f32r matmuls, fp16 drain, DMA
        with (
            tc.tile_pool(name="psl", bufs=4, space="PSUM") as psl,
            tc.tile_pool(name="stage", bufs=3) as stpool,
            tc.tile_pool(name="statp", bufs=2) as statpool,
        ):
            # greedy least-busy drain assignment (model costs per engine, ns)
            # (GpSimd cannot access PSUM on TRN2, so only DVE + ACT drain)
            dcost = {
                "v": lambda w: w * 1.042 + 125.0,
                "a": lambda w: w * 0.833 + 185.0,
            }
            dbusy = {"v": 0.0, "a": 0.0}
            for blk in range(ROWS // 128):
                statq = statpool.tile([128, 128], f32r, tag="statq")
                for qb in (0, 64):
                    nc.vector.tensor_copy(
                        statq[qb:qb + 11, :],
                        hsb[:, W + 128 * blk: W + 128 * blk + 128])
                for q in range(NQ):
                    stat = statq[64 * q:64 * q + 11, :]
                    chunks = _chunks_for_q(q)
                    stage = None
                    soff = 0
                    scol = 0
                    for ci in range(0, len(chunks), 2):
                        pair = chunks[ci:ci + 2]
                        pw = sum(w for _, w in pair)
                        ps = psl.tile([128, 1024], f32, tag="lg")
                        po = 0
                        for c0, w in pair:
                            nc.tensor.matmul(
                                ps[:, po:po + w], lhsT=stat,
                                rhs=wout_sb[64 * q:64 * q + 11, c0:c0 + w],
                                start=True, stop=True)
                            po += w
                        if stage is None:
                            stage = stpool.tile([128, STG], fp16, tag="stg")
                            soff = 0
                            scol = pair[0][0]
                        eng = min(dbusy, key=lambda e: dbusy[e] + dcost[e](pw))
                        dbusy[eng] += dcost[eng](pw)
                        dst = stage[:, soff:soff + pw]
                        if eng == "v":
                            nc.vector.tensor_copy(dst, ps[:, 0:pw])
                        elif eng == "a":
                            nc.scalar.copy(dst, ps[:, 0:pw])
                        else:
                            nc.gpsimd.tensor_copy(dst, ps[:, 0:pw])
                        soff += pw
                        if soff + 1024 > STG or ci + 2 >= len(chunks):
                            col = q * QD + scol
                            nc.sync.dma_start(
                                out[ts(blk, 128), col:col + soff],
                                stage[:, 0:soff])
                            stage = None


def build_program_real(variant="main"):
    nc = bacc.Bacc("TRN2", target_bir_lowering=False, debug=False,
                   enable_asserts=False)
    emb_d = nc.dram_tensor("emb", [V + 1, E], f32, kind="ExternalInput")
    idx_d = nc.dram_tensor("idx", [128, CB], i32, kind="ExternalInput")
    wih_d = nc.dram_tensor("wih", [E, GP], f32, kind="ExternalInput")
    whh_d = nc.dram_tensor("whh", [H, GP], f32r, kind="ExternalInput")
    b106_d = nc.dram_tensor("b106", [GP, 1], f32, kind="ExternalInput")
    wout_d = nc.dram_tensor("wout", [22, QD], f32r, kind="ExternalInput")
    out_d = nc.dram_tensor("out", [ROWS, NQ * QD], fp16, kind="ExternalOutput")
    phases = {"fs": ("front", "sweep"), "lo": ("logits",),
              "sw": ("sweep",)}.get(variant, ("front", "sweep", "logits"))
    dbg = None
    if variant == "debug":
        dbg = {
            "xg": nc.dram_tensor("dbg_xg", [GP, NR], f32,
                                 kind="ExternalOutput").ap(),
            "hsb": nc.dram_tensor("dbg_hsb", [11, NR], f32,
                                  kind="ExternalOutput").ap(),
        }

    with tile.TileContext(nc) as tc:
        _tile_kernel(tc, nc, emb_d.ap(), idx_d.ap(), wih_d.ap(), whh_d.ap(),
                     b106_d.ap(), wout_d.ap(), out_d.ap(), dbg=dbg,
                     phases=phases)
    nc.compile()
    return nc


def prep_host(inputs):
    """Shared (core-independent) prepped arrays + per-core index tables."""
    x = np.asarray(inputs["x"]).astype(np.int64)
    embedding = np.asarray(inputs["embedding"], dtype=np.float32)
    w_ih = np.asarray(inputs["w_ih"], dtype=np.float32)
    w_hh = np.asarray(inputs["w_hh"], dtype=np.float32)
    b_ih = np.asarray(inputs["b_ih"], dtype=np.float32)
    b_hh = np.asarray(inputs["b_hh"], dtype=np.float32)
    W_out = np.asarray(inputs["W_out"], dtype=np.float32)
    b_out = np.asarray(inputs["b_out"], dtype=np.float32)

    p = GATE_PERM
    w_ih_p = w_ih[p]                           # [40, E] in i,f,o,g order
    bias_p = (b_ih + b_hh)[p]
    whh_p = w_hh[p]                            # [40, H]

    # scatter the 4 gates to partition bases 0/32/64/96
    wih106 = np.zeros((E, GP), np.float32)
    b106 = np.zeros((GP, 1), np.float32)
    whh106 = np.zeros((H, GP), np.float32)
    for g in range(4):
        o = GOFF[g]
        wih106[:, o:o + H] = w_ih_p[g * H:(g + 1) * H].T
        b106[o:o + H, 0] = bias_p[g * H:(g + 1) * H]
        whh106[:, o:o + H] = whh_p[g * H:(g + 1) * H].T

    # Padding row V: w_ih @ v = -(b_ih + b_hh)  => xg row == 0 for padded rows
    v, *_ = np.linalg.lstsq(w_ih.astype(np.float64),
                            -(b_ih + b_hh).astype(np.float64), rcond=None)
    emb_aug = np.concatenate([embedding, v[None, :].astype(np.float32)], axis=0)

    # wout: rows 0:10 / 11:21 are W_out^T column halves, rows 10/21 the bias
    woutp = np.zeros((22, QD), np.float32)
    woutp[0:10, :] = W_out[0:QD].T
    woutp[10, :] = b_out[0:QD]
    woutp[11:21, 0:O - QD] = W_out[QD:O].T
    woutp[21, 0:O - QD] = b_out[QD:O]

    idx_cores = []
    for c in range(NCORES):
        j = np.arange(NR)
        g_r = c * ROWS + j - W
        val = np.where((g_r < 0) | (j >= NUSE), V, x[np.clip(g_r, 0, T - 1)])
        idx_cores.append(val.reshape(CB, 128).T.astype(np.int32))

    shared = {
        "emb": emb_aug,
        "wih": wih106,
        "whh": whh106,
        "b106": b106,
        "wout": woutp,
    }
    return shared, idx_cores


def in_maps_for(inputs):
    shared, idx_cores = prep_host(inputs)
    return [{**shared, "idx": idx_cores[c]} for c in range(NCORES)]


_EXEC_CACHE = {}


def _get_exec(variant="main"):
    """Build (once) the compiled 8-core PJRT executable and metadata."""
    if variant in _EXEC_CACHE:
        return _EXEC_CACHE[variant]

    import jax
    from jax.sharding import Mesh, PartitionSpec, NamedSharding
    try:
        from jax.experimental.shard_map import shard_map
    except ImportError:
        from jax import shard_map
    from concourse import bass2jax

    bass2jax.install_neuronx_cc_hook()
    nc = build_program_real(variant)

    pname = nc.partition_id_tensor.name if nc.partition_id_tensor else None
    in_names, out_names, out_avals = [], [], []
    for alloc in nc.m.functions[0].allocations:
        if not isinstance(alloc, mybir.MemoryLocationSet):
            continue
        name = alloc.memorylocations[0].name
        if alloc.kind == "ExternalInput":
            if name != pname:
                in_names.append(name)
        elif alloc.kind == "ExternalOutput":
            out_names.append(name)
            out_avals.append(jax.core.ShapedArray(
                tuple(alloc.tensor_shape), mybir.dt.np(alloc.dtype)))
    n_params = len(in_names)
    all_names = in_names + out_names + ([pname] if pname else [])

    def _body(*args):
        operands = list(args)
        if pname is not None:
            operands.append(bass2jax.partition_id_tensor())
        outs = bass2jax._bass_exec_p.bind(
            *operands,
            out_avals=tuple(out_avals),
            in_names=tuple(all_names),
            out_names=tuple(out_names),
            lowering_input_output_aliases=(),
            sim_require_finite=False,
            sim_require_nnan=False,
            nc=nc,
        )
        return tuple(outs)

    devices = jax.devices()[:NCORES]
    mesh = Mesh(np.asarray(devices), ("core",))
    spec_in = (PartitionSpec("core"),) * (n_params + len(out_names))
    spec_out = (PartitionSpec("core"),) * len(out_names)
    donate = tuple(range(n_params, n_params + len(out_names)))
    fn = jax.jit(
        shard_map(_body, mesh=mesh, in_specs=spec_in, out_specs=spec_out,
                  check_rep=False),
        donate_argnums=donate, keep_unused=True)

    res = {
        "jax": jax, "mesh": mesh, "NamedSharding": NamedSharding,
        "PartitionSpec": PartitionSpec, "fn": fn, "nc": nc,
        "in_names": in_names, "out_names": out_names, "out_avals": out_avals,
        "devices": devices,
    }
    _EXEC_CACHE[variant] = res
    return res


def _place_inputs(ex, in_maps):
    """Transfer per-core input shards to the 8 devices, return global arrays."""
    jax = ex["jax"]
    NamedSharding, PartitionSpec = ex["NamedSharding"], ex["PartitionSpec"]
    sharding = NamedSharding(ex["mesh"], PartitionSpec("core"))
    placed = []
    for name in ex["in_names"]:
        shards = [np.asarray(in_maps[c][name]) for c in range(NCORES)]
        per_dev = [jax.device_put(s, d) for s, d in zip(shards, ex["devices"])]
        gshape = (NCORES * shards[0].shape[0],) + shards[0].shape[1:]
        placed.append(jax.make_array_from_single_device_arrays(
            gshape, sharding, per_dev))
    jax.block_until_ready(placed)
    return placed, sharding


def _zero_outs(ex, sharding):
    import jax.numpy as jnp
    outs = []
    for av in ex["out_avals"]:
        gshape = (NCORES * av.shape[0],) + av.shape[1:]
        outs.append(jnp.zeros(gshape, av.dtype, device=sharding))
    ex["jax"].block_until_ready(outs)
    return outs


def run_hw(inputs, time_iters=0, variant=None):
    """Run on the 8 NeuronCores. Returns (full_output, wall_times_s)."""
    if variant is None:
        variant = os.environ.get("KERNEL_VARIANT", "main")
    ex = _get_exec(variant)
    jax = ex["jax"]
    in_maps = in_maps_for(inputs)
    placed, sharding = _place_inputs(ex, in_maps)

    zouts = _zero_outs(ex, sharding)
    res = ex["fn"](*placed, *zouts)
    jax.block_until_ready(res)
    out_global = np.asarray(res[0])          # [8*512, NQ*QD] fp16

    times = []
    for _ in range(time_iters):
        zouts = _zero_outs(ex, sharding)
        t0 = time.perf_counter()
        r = ex["fn"](*placed, *zouts)
        jax.block_until_ready(r)
        times.append(time.perf_counter() - t0)

    full = out_global[:, :O].astype(np.float32).reshape(T, 1, O)
    return full, times


def kernel(**inputs):
    out, _ = run_hw(inputs, time_iters=0)
    return out


# ---------------------------------------------------------------- dev helpers

def sim_check(inputs, core=0, variant="main"):
    """Run core `core`'s program in CoreSim, return its [512, NQ*QD] output."""
    from concourse.bass_interp import CoreSim
    nc = build_program_real(variant)
    sim = CoreSim(nc, trace=False, require_finite=False, require_nnan=False)
    in_maps = in_maps_for(inputs)
    for name, arr in in_maps[core].items():
        try:
            sim.tensor(name)[:] = arr
        except KeyError:
            pass
    sim.simulate(check_with_hw=False)
    return np.array(sim.tensor("out"))


def timeline(variant="main"):
    from concourse.timeline_sim import TimelineSim
    nc = build_program_real(variant)
    tl = TimelineSim(nc, trace=False)
    tl.simulate()
    return tl


def probe_floor(iters=5):
    """Wall-time floor of the 8-core dispatch path using a trivial NEFF."""
    import jax
    from jax.sharding import Mesh, PartitionSpec, NamedSharding
    try:
        from jax.experimental.shard_map import shard_map
    except ImportError:
        from jax import shard_map
    from concourse import bass2jax
    bass2jax.install_neuronx_cc_hook()

    nc = bacc.Bacc("TRN2", target_bir_lowering=False, debug=False,
                   enable_asserts=False)
    pin = nc.dram_tensor("pin", [128, 128], f32, kind="ExternalInput")
    pout = nc.dram_tensor("pout", [128, 128], f32, kind="ExternalOutput")
    with tile.TileContext(nc) as tc:
        with tc.tile_pool(name="p", bufs=1) as pool:
            t = pool.tile([128, 128], f32, tag="t")
            nc.sync.dma_start(t[:], pin.ap()[:])
            nc.sync.dma_start(pout.ap()[:], t[:])
    nc.compile()

    pname = nc.partition_id_tensor.name if nc.partition_id_tensor else None
    all_names = ["pin", "pout"] + ([pname] if pname else [])

    def _body(a, z):
        ops = [a, z]
        if pname is not None:
            ops.append(bass2jax.partition_id_tensor())
        return tuple(bass2jax._bass_exec_p.bind(
            *ops, out_avals=(jax.core.ShapedArray((128, 128), np.float32),),
            in_names=tuple(all_names), out_names=("pout",),
            lowering_input_output_aliases=(),
            sim_require_finite=False, sim_require_nnan=False, nc=nc))

    devices = jax.devices()[:NCORES]
    mesh = Mesh(np.asarray(devices), ("core",))
    sharding = NamedSharding(mesh, PartitionSpec("core"))
    fn = jax.jit(shard_map(_body, mesh=mesh,
                           in_specs=(PartitionSpec("core"),) * 2,
                           out_specs=(PartitionSpec("core"),),
                           check_rep=False), keep_unused=True)
    import jax.numpy as jnp
    a = jax.device_put(np.zeros((NCORES * 128, 128), np.float32), sharding)
    z = jnp.zeros((NCORES * 128, 128), np.float32, device=sharding)
    jax.block_until_ready([a, z])
    r = fn(a, z); jax.block_until_ready(r)   # warm

    def timed(reps):
        best = float("inf")
        for _ in range(iters):
            t0 = time.perf_counter()
            r = None
            for _ in range(reps):
                r = fn(a, z)
            jax.block_until_ready(r)
            best = min(best, time.perf_counter() - t0)
        return best

    w1 = timed(1)
    wk = timed(50)
    return (wk - w1) / 49.0, wk, w1


def run_hw_async(inputs, k=50, iters=3, variant="main"):
    """Per-exec time via async pipelining: submit k executions without
    intermediate blocking; marginal cost per call ~= device exec time if the
    runtime queues them. Returns (per_exec_s, wall_k, wall_1)."""
    import jax
    from jax.sharding import PartitionSpec
    try:
        from jax.experimental.shard_map import shard_map
    except ImportError:
        from jax import shard_map
    from concourse import bass2jax
    ex = _get_exec(variant)
    nc = ex["nc"]
    pname = nc.partition_id_tensor.name if nc.partition_id_tensor else None
    in_names, out_names, out_avals = ex["in_names"], ex["out_names"], ex["out_avals"]
    all_names = in_names + out_names + ([pname] if pname else [])

    def _body(*args):
        ops = list(args)
        if pname is not None:
            ops.append(bass2jax.partition_id_tensor())
        return tuple(bass2jax._bass_exec_p.bind(
            *ops, out_avals=tuple(out_avals), in_names=tuple(all_names),
            out_names=tuple(out_names), lowering_input_output_aliases=(),
            sim_require_finite=False, sim_require_nnan=False, nc=nc))

    nin = len(in_names) + len(out_names)
    fn = jax.jit(shard_map(_body, mesh=ex["mesh"],
                           in_specs=(PartitionSpec("core"),) * nin,
                           out_specs=(PartitionSpec("core"),) * len(out_names),
                           check_rep=False), keep_unused=True)  # no donation

    in_maps = in_maps_for(inputs)
    placed, sharding = _place_inputs(ex, in_maps)
    zouts = _zero_outs(ex, sharding)
    r = fn(*placed, *zouts); jax.block_until_ready(r)   # warm

    def timed(reps):
        best = float("inf")
        for _ in range(iters):
            t0 = time.perf_counter()
            r = None
            for _ in range(reps):
                r = fn(*placed, *zouts)
            jax.block_until_ready(r)
            best = min(best, time.perf_counter() - t0)
        return best

    w1 = timed(1)
    wk = timed(k)
    return (wk - w1) / (k - 1), wk, w1
